# revision 3
# baseline (speedup 1.0000x reference)
"""Trainium2 Bass kernel for nn_ProbabilisticMap.

Math: for each (batch b, curve-sample t) the reference computes a 2D Gaussian
density over a 64x64 pixel grid:
    out[b,x,y,t] = exp(-0.5 * q) / sqrt((2pi)^2 det)
    q = (p - mean)^T inv(cov) (p - mean),   p = (x, y)
with mean/cov the Bernstein(num_cps[b])-weighted combination of control-point
means/covariances.

Kernel strategy (pure data-parallel over batch, 16 examples per core):
  Small stage (per-(b,t) scalars, ~1k values):
    - Bernstein weights w[(b,k), t] are a row-select from a constant table
      W2[(n,k), t]; the select is a one-hot (over n*8+k) matmul on the PE.
    - mean/cov contractions over k are matmuls against a constant
      block-diagonal selector BD[(b,k), b].
    - 2x2 inverse/det/quadratic-form coefficients on the vector engine.
    - The density is rewritten in pixel-monomial form:
        q' = A x^2 + B xy + C y^2 + D x + E y + F'
      with F' absorbing log((2pi)^2 det) so out = exp(-0.5 q') needs no
      per-column bias.
  Big stage (33.5M outputs):
    - q' = G^T @ Coef : G is the constant [6, 4096] monomial basis of the
      pixel grid, Coef the [6, (b,t)] coefficients -> PE matmul into PSUM.
    - out = exp(-0.5 q') : one scalar-engine activation PSUM->SBUF.
    - DMA [128 pix, 16 b, 64 t] tiles to HBM (t innermost matches layout).
"""

import math

import numpy as np

NCORES = 8
BATCH = 128
B_LOC = BATCH // NCORES  # 16
K = 8
T = 64
W = 64
H = 64
NPIX = W * H  # 4096
NCHUNK = NPIX // 128  # 32
J = 72  # one-hot rows: degree n in 0..8 (num_cps in 1..9), j = n*8 + k

_CACHE = {}


def _constants():
    t = np.linspace(0.0, 1.0, T)
    # Bernstein table: W2[n*8+k, t] = C(n,k) t^k (1-t)^(n-k), 0 for k > n
    w2 = np.zeros((J, T), dtype=np.float64)
    for n in range(J // K):
        for k in range(K):
            if k <= n:
                w2[n * K + k] = math.comb(n, k) * t**k * (1.0 - t) ** (n - k)
    # pixel monomial basis; pix = x*64 + y
    pix = np.arange(NPIX)
    x = (pix // H).astype(np.float64)
    y = (pix % H).astype(np.float64)
    g = np.stack([x * x, x * y, y * y, x, y, np.ones_like(x)])  # [6, 4096]
    # block-diagonal selector: BD[(b,k), b'] = (b == b')
    bd = np.zeros((B_LOC * K, B_LOC), dtype=np.float64)
    for p in range(B_LOC * K):
        bd[p, p // K] = 1.0
    km8 = (np.arange(BATCH) % K - K).astype(np.float64).reshape(1, BATCH)
    iota_j = np.arange(J, dtype=np.float64).reshape(J, 1)
    ones_j = np.ones((1, J), dtype=np.float64)
    return {
        "W2": w2.astype(np.float32),
        "G": g.astype(np.float32),
        "BD": bd.astype(np.float32),
        "KM8": km8.astype(np.float32),
        "IOTJ": iota_j.astype(np.float32),
        "ONESJ": ones_j.astype(np.float32),
    }


def _build_nc():
    import concourse.bacc as bacc
    import concourse.bass as bass
    import concourse.tile as tile
    from concourse import mybir

    f32 = mybir.dt.float32
    alu = mybir.AluOpType
    act = mybir.ActivationFunctionType

    nc = bacc.Bacc(
        "TRN2",
        target_bir_lowering=False,
        debug=False,
        enable_asserts=True,
        num_devices=NCORES,
    )

    cp_xy_d = nc.dram_tensor("cp_xy", [128, 2], f32, kind="ExternalInput")
    cov_d = nc.dram_tensor("cov_abc", [128, 4], f32, kind="ExternalInput")
    nrep_d = nc.dram_tensor("n_rep", [1, 128], f32, kind="ExternalInput")
    out_d = nc.dram_tensor("out", [B_LOC, NCHUNK, 128, T], f32, kind="ExternalOutput")

    consts = _constants()
    w2_d = nc.inline_tensor(consts["W2"], name="w2_tbl")
    g_d = nc.inline_tensor(consts["G"], name="g_tbl")
    bd_d = nc.inline_tensor(consts["BD"], name="bd_tbl")
    km8_d = nc.inline_tensor(consts["KM8"], name="km8_tbl")
    iotj_d = nc.inline_tensor(consts["IOTJ"], name="iotj_tbl")

    log2pi2 = float(2.0 * math.log(2.0 * math.pi))

    with tile.TileContext(nc) as tc:
        with (
            tc.tile_pool(name="const", bufs=1) as cpool,
            tc.tile_pool(name="small", bufs=1) as sm,
            tc.tile_pool(name="spsum", bufs=1, space="PSUM") as sps,
            tc.tile_pool(name="bpsum", bufs=4, space="PSUM") as bps,
            tc.tile_pool(name="bigout", bufs=3) as big,
        ):
            # ---- constant / input loads ----
            w2_s = cpool.tile([J, T], f32, tag="w2")
            nc.sync.dma_start(out=w2_s, in_=w2_d.ap())
            g_s = cpool.tile([6, NPIX], f32, tag="g")
            nc.sync.dma_start(out=g_s, in_=g_d.ap())
            bd_s = cpool.tile([128, B_LOC], f32, tag="bd")
            nc.sync.dma_start(out=bd_s, in_=bd_d.ap())
            km8_s = cpool.tile([1, 128], f32, tag="km8")
            nc.sync.dma_start(out=km8_s, in_=km8_d.ap())
            iotj_s = cpool.tile([J, 1], f32, tag="iotj")
            nc.sync.dma_start(out=iotj_s, in_=iotj_d.ap())
            cxy_s = cpool.tile([128, 2], f32, tag="cxy")
            nc.sync.dma_start(out=cxy_s, in_=cp_xy_d.ap())
            cov_s = cpool.tile([128, 4], f32, tag="cov")
            nc.sync.dma_start(out=cov_s, in_=cov_d.ap())
            nrep_s = cpool.tile([1, 128], f32, tag="nrep")
            nc.sync.dma_start(out=nrep_s, in_=nrep_d.ap())

            # ---- small stage: per-(b,t) coefficients ----
            # j = (num_cps-1)*8 + k = 8*n + (k-8)
            jr = sm.tile([1, 128], f32, tag="jr")
            nc.vector.tensor_scalar(
                out=jr, in0=nrep_s, scalar1=8.0, scalar2=None, op0=alu.mult
            )
            nc.vector.tensor_tensor(out=jr, in0=jr, in1=km8_s, op=alu.add)

            # broadcast j across J partitions via K=1 matmul with a ones column
            ones_col = cpool.tile([1, J], f32, tag="onesj")
            nc.vector.memset(ones_col, 1.0)
            jb_ps = sps.tile([J, 128], f32, tag="jb")
            nc.tensor.matmul(jb_ps, lhsT=ones_col, rhs=jr, start=True, stop=True)

            onehot = sm.tile([J, 128], f32, tag="onehot")
            nc.vector.tensor_scalar(
                out=onehot, in0=jb_ps, scalar1=iotj_s, scalar2=None, op0=alu.is_equal
            )

            # Bernstein weights w[(b,k), t] and w^2
            w_ps = sps.tile([128, T], f32, tag="wps")
            nc.tensor.matmul(w_ps, lhsT=onehot, rhs=w2_s, start=True, stop=True)
            w_s = sm.tile([128, T], f32, tag="w")
            nc.vector.tensor_copy(w_s, w_ps)
            w2q_s = sm.tile([128, T], f32, tag="w2q")
            nc.scalar.square(w2q_s, w_ps)

            # weighted per-(b,k) rows, then contract k via BD matmuls -> [b, t]
            def bd_contract(src, col, scale_tile, tag):
                wm = sm.tile([128, T], f32, tag="wm_" + tag, name="wm_" + tag)
                nc.vector.tensor_scalar(
                    out=wm,
                    in0=src,
                    scalar1=scale_tile[:, col : col + 1],
                    scalar2=None,
                    op0=alu.mult,
                )
                ps = sps.tile([B_LOC, T], f32, tag="ps", name="ps_" + tag, bufs=2)
                nc.tensor.matmul(ps, lhsT=bd_s, rhs=wm, start=True, stop=True)
                res = sm.tile([B_LOC, T], f32, tag="r_" + tag, name="r_" + tag)
                nc.vector.tensor_copy(res, ps)
                return res

            mx = bd_contract(w_s, 0, cxy_s, "mx")
            my = bd_contract(w_s, 1, cxy_s, "my")
            ca = bd_contract(w2q_s, 0, cov_s, "ca")
            cb = bd_contract(w2q_s, 1, cov_s, "cb")
            cc = bd_contract(w2q_s, 3, cov_s, "cc")

            def vt(tag):
                return sm.tile([B_LOC, T], f32, tag=tag, name=tag)

            mul, add, sub = alu.mult, alu.add, alu.subtract

            def tt(out, a, b, op):
                nc.vector.tensor_tensor(out=out, in0=a, in1=b, op=op)

            det = vt("det")
            tmp = vt("tmp")
            tt(det, ca, cc, mul)
            tt(tmp, cb, cb, mul)
            tt(det, det, tmp, sub)
            rdet = vt("rdet")
            nc.vector.reciprocal(rdet, det)

            i00 = vt("i00")  # = A coefficient (x^2)
            i01n = vt("i01n")  # +b/det (true inv offdiag is the negative)
            i11 = vt("i11")  # = C coefficient (y^2)
            tt(i00, cc, rdet, mul)
            tt(i01n, cb, rdet, mul)
            tt(i11, ca, rdet, mul)

            bq = vt("bq")  # B coefficient (xy) = -2*i01n
            nc.vector.tensor_scalar(
                out=bq, in0=i01n, scalar1=-2.0, scalar2=None, op0=mul
            )

            d1 = vt("d1")
            d2 = vt("d2")
            tt(d1, i00, mx, mul)  # i00*Mx
            tt(d2, i01n, my, mul)  # (b/det)*My
            dc = vt("dc")  # D coefficient (x) = 2*(d2 - d1)
            tt(dc, d2, d1, sub)
            nc.vector.tensor_scalar(out=dc, in0=dc, scalar1=2.0, scalar2=None, op0=mul)

            e1 = vt("e1")
            e2 = vt("e2")
            tt(e1, i11, my, mul)
            tt(e2, i01n, mx, mul)
            ec = vt("ec")  # E coefficient (y) = 2*(e2 - e1)
            tt(ec, e2, e1, sub)
            nc.vector.tensor_scalar(out=ec, in0=ec, scalar1=2.0, scalar2=None, op0=mul)

            # F' = i00 Mx^2 + i11 My^2 - 2 (b/det) Mx My + ln(det) + 2 ln(2pi)
            f1 = vt("f1")
            f2 = vt("f2")
            f3 = vt("f3")
            tt(f1, d1, mx, mul)
            tt(f2, e1, my, mul)
            tt(f3, d2, mx, mul)
            fc = vt("fc")
            tt(fc, f1, f2, add)
            nc.vector.tensor_scalar(
                out=f3, in0=f3, scalar1=-2.0, scalar2=None, op0=mul
            )
            tt(fc, fc, f3, add)
            ld = vt("ld")
            nc.scalar.activation(ld, det, func=act.Ln)
            tt(fc, fc, ld, add)
            nc.vector.tensor_scalar(
                out=fc, in0=fc, scalar1=log2pi2, scalar2=None, op0=add
            )

            # ---- assemble Coef [6, (b,t)] via partition-collapse DMAs ----
            coef = cpool.tile([6, B_LOC * T], f32, tag="coef")
            for c, src in enumerate([i00, bq, i11, dc, ec, fc]):
                nc.sync.dma_start(out=coef[c : c + 1, :], in_=src[:])

            # ---- big stage ----
            out_ap = out_d.ap()
            for chunk in range(NCHUNK):
                o = big.tile([128, B_LOC, T], f32, tag="o")
                for bg in range(2):
                    q_ps = bps.tile([128, 512], f32, tag="q")
                    nc.tensor.matmul(
                        q_ps,
                        lhsT=g_s[:, chunk * 128 : (chunk + 1) * 128],
                        rhs=coef[:, bg * 512 : (bg + 1) * 512],
                        start=True,
                        stop=True,
                    )
                    nc.scalar.activation(
                        o[:, bg * 8 : (bg + 1) * 8, :],
                        q_ps[:].rearrange("p (b t) -> p b t", b=8),
                        func=act.Exp,
                        scale=-0.5,
                    )
                dst = out_ap[:, chunk].rearrange("b p t -> p b t")
                nc.sync.dma_start(out=dst, in_=o[:])

    nc.compile()
    return nc


def _get_nc():
    if "nc" not in _CACHE:
        _CACHE["nc"] = _build_nc()
    return _CACHE["nc"]


def make_in_maps(cp_means, num_cps, cp_covariances):
    cp_means = np.asarray(cp_means, dtype=np.float32)
    cp_covariances = np.asarray(cp_covariances, dtype=np.float32)
    num_cps = np.asarray(num_cps)
    in_maps = []
    for c in range(NCORES):
        bsl = slice(c * B_LOC, (c + 1) * B_LOC)
        cxy = cp_means[:, bsl, :].transpose(1, 0, 2).reshape(128, 2)
        cab = cp_covariances[:, bsl].transpose(1, 0, 2, 3).reshape(128, 4)
        nrep = np.repeat(num_cps[bsl].astype(np.float32), K).reshape(1, 128)
        in_maps.append(
            {
                "cp_xy": np.ascontiguousarray(cxy),
                "cov_abc": np.ascontiguousarray(cab),
                "n_rep": np.ascontiguousarray(nrep),
            }
        )
    return in_maps


def kernel(cp_means, num_cps, cp_covariances):
    from concourse.bass_utils import run_bass_kernel_spmd

    nc = _get_nc()
    in_maps = make_in_maps(cp_means, num_cps, cp_covariances)
    res = run_bass_kernel_spmd(nc, in_maps, list(range(NCORES))).results
    out = np.concatenate(
        [res[i]["out"].reshape(B_LOC, W, H, T) for i in range(NCORES)], axis=0
    )
    return np.ascontiguousarray(out, dtype=np.float32)


# revision 10
# speedup vs baseline: 1.1564x; 1.1564x over previous
"""Trainium2 Bass kernel for nn_ProbabilisticMap.

Math: for each (batch b, curve-sample t) the reference computes a 2D Gaussian
density over a 64x64 pixel grid:
    out[b,x,y,t] = exp(-0.5 * q) / sqrt((2pi)^2 det)
    q = (p - mean)^T inv(cov) (p - mean),   p = (x, y)
with mean/cov the Bernstein(num_cps[b])-weighted combination of control-point
means/covariances.

Kernel strategy (pure data-parallel over batch, 16 examples per core):
  Small stage (per-(b,t) scalars, ~1k values):
    - Bernstein weights w[(b,k), t] are a row-select from a constant table
      W2[(n,k), t]; the select is a one-hot (over n*8+k) matmul on the PE.
    - mean/cov contractions over k are matmuls against a constant
      block-diagonal selector BD[(b,k), b].
    - 2x2 inverse/det/quadratic-form coefficients on the vector engine.
    - The density is rewritten in pixel-monomial form:
        q' = A x^2 + B xy + C y^2 + D x + E y + F'
      with F' absorbing log((2pi)^2 det) so out = exp(-0.5 q') needs no
      per-column bias.
  Big stage (33.5M outputs):
    - q' = G^T @ Coef : G is the constant [6, 4096] monomial basis of the
      pixel grid, Coef the [6, (b,t)] coefficients -> PE matmul into PSUM.
    - out = exp(-0.5 q') : one scalar-engine activation PSUM->SBUF.
    - DMA [128 pix, 16 b, 64 t] tiles to HBM (t innermost matches layout).
"""

import math

import numpy as np

NCORES = 8
BATCH = 128
B_LOC = BATCH // NCORES  # 16
K = 8
T = 64
W = 64
H = 64
NPIX = W * H  # 4096
NCHUNK = NPIX // 128  # 32
J = 72  # one-hot rows: degree n in 0..8 (num_cps in 1..9), j = n*8 + k

_CACHE = {}


def _constants():
    t = np.linspace(0.0, 1.0, T)
    # Bernstein table: W2[n*8+k, t] = C(n,k) t^k (1-t)^(n-k), 0 for k > n
    w2 = np.zeros((J, T), dtype=np.float64)
    for n in range(J // K):
        for k in range(K):
            if k <= n:
                w2[n * K + k] = math.comb(n, k) * t**k * (1.0 - t) ** (n - k)
    # pixel monomial basis; pix = x*64 + y
    pix = np.arange(NPIX)
    x = (pix // H).astype(np.float64)
    y = (pix % H).astype(np.float64)
    g = np.stack([x * x, x * y, y * y, x, y, np.ones_like(x)])  # [6, 4096]
    # exact bf16 split of G: integer entries <= 3969 fit in Gh + Gl exactly
    import ml_dtypes

    bf16 = ml_dtypes.bfloat16
    gh = g.astype(np.float32).astype(bf16)
    gl = (g.astype(np.float32) - gh.astype(np.float32)).astype(bf16)
    assert np.all(gh.astype(np.float64) + gl.astype(np.float64) == g)
    gstack = np.concatenate([gh, gh, gh, gl, gl, gl], axis=0)  # [36, 4096]
    # block-diagonal selector: BD[(b,k), b'] = (b == b')
    bd = np.zeros((B_LOC * K, B_LOC), dtype=np.float64)
    for p in range(B_LOC * K):
        bd[p, p // K] = 1.0
    km8 = (np.arange(BATCH) % K - K).astype(np.float64).reshape(1, BATCH)
    iota_j = np.arange(J, dtype=np.float64).reshape(J, 1)
    ones_j = np.ones((1, J), dtype=np.float64)
    return {
        "W2": w2.astype(np.float32),
        "GSTACK": gstack,
        "BD": bd.astype(np.float32),
        "KM8": km8.astype(np.float32),
        "IOTJ": iota_j.astype(np.float32),
        "ONESJ": ones_j.astype(np.float32),
    }


def _build_nc():
    import concourse.bacc as bacc
    import concourse.bass as bass
    import concourse.tile as tile
    from concourse import mybir

    f32 = mybir.dt.float32
    alu = mybir.AluOpType
    act = mybir.ActivationFunctionType

    nc = bacc.Bacc(
        "TRN2",
        target_bir_lowering=False,
        debug=False,
        enable_asserts=True,
        num_devices=NCORES,
    )

    cp_xy_d = nc.dram_tensor("cp_xy", [128, 2], f32, kind="ExternalInput")
    cov_d = nc.dram_tensor("cov_abc", [128, 4], f32, kind="ExternalInput")
    nrep_d = nc.dram_tensor("n_rep", [1, 128], f32, kind="ExternalInput")
    out_d = nc.dram_tensor("out", [B_LOC, NCHUNK, 128, T], f32, kind="ExternalOutput")

    consts = _constants()
    w2_d = nc.inline_tensor(consts["W2"], name="w2_tbl")
    g_d = nc.inline_tensor(consts["GSTACK"], name="gstack_tbl")
    bd_d = nc.inline_tensor(consts["BD"], name="bd_tbl")
    km8_d = nc.inline_tensor(consts["KM8"], name="km8_tbl")
    iotj_d = nc.inline_tensor(consts["IOTJ"], name="iotj_tbl")

    log2pi2 = float(2.0 * math.log(2.0 * math.pi))

    with tile.TileContext(nc) as tc:
        with (
            tc.tile_pool(name="const", bufs=1) as cpool,
            tc.tile_pool(name="small", bufs=1) as sm,
            tc.tile_pool(name="bigout", bufs=3) as big,
        ):
            sps = tc.alloc_tile_pool(name="spsum", bufs=1, space="PSUM")
            # ---- constant / input loads ----
            w2_s = cpool.tile([J, T], f32, tag="w2")
            nc.sync.dma_start(out=w2_s, in_=w2_d.ap())
            bf = mybir.dt.bfloat16
            g_s = cpool.tile([36, NPIX], bf, tag="g")
            nc.sync.dma_start(out=g_s, in_=g_d.ap())
            bd_s = cpool.tile([128, B_LOC], f32, tag="bd")
            nc.sync.dma_start(out=bd_s, in_=bd_d.ap())
            km8_s = cpool.tile([1, 128], f32, tag="km8")
            nc.sync.dma_start(out=km8_s, in_=km8_d.ap())
            iotj_s = cpool.tile([J, 1], f32, tag="iotj")
            nc.sync.dma_start(out=iotj_s, in_=iotj_d.ap())
            cxy_s = cpool.tile([128, 2], f32, tag="cxy")
            nc.sync.dma_start(out=cxy_s, in_=cp_xy_d.ap())
            cov_s = cpool.tile([128, 4], f32, tag="cov")
            nc.sync.dma_start(out=cov_s, in_=cov_d.ap())
            nrep_s = cpool.tile([1, 128], f32, tag="nrep")
            nc.sync.dma_start(out=nrep_s, in_=nrep_d.ap())

            # ---- small stage: per-(b,t) coefficients ----
            # j = (num_cps-1)*8 + k = 8*n + (k-8)
            jr = sm.tile([1, 128], f32, tag="jr")
            nc.vector.tensor_scalar(
                out=jr, in0=nrep_s, scalar1=8.0, scalar2=None, op0=alu.mult
            )
            nc.vector.tensor_tensor(out=jr, in0=jr, in1=km8_s, op=alu.add)

            # broadcast j across J partitions via K=1 matmul with a ones column
            ones_col = cpool.tile([1, J], f32, tag="onesj")
            nc.vector.memset(ones_col, 1.0)
            jb_ps = sps.tile([J, 128], f32, tag="jb")
            nc.tensor.matmul(jb_ps, lhsT=ones_col, rhs=jr, start=True, stop=True)

            onehot = sm.tile([J, 128], f32, tag="onehot")
            nc.vector.tensor_scalar(
                out=onehot, in0=jb_ps, scalar1=iotj_s, scalar2=None, op0=alu.is_equal
            )

            # Bernstein weights w[(b,k), t] and w^2
            w_ps = sps.tile([128, T], f32, tag="wps")
            nc.tensor.matmul(w_ps, lhsT=onehot, rhs=w2_s, start=True, stop=True)
            w_s = sm.tile([128, T], f32, tag="w")
            nc.vector.tensor_copy(w_s, w_ps)
            w2q_s = sm.tile([128, T], f32, tag="w2q")
            nc.scalar.square(w2q_s, w_ps)

            # weighted per-(b,k) rows, then contract k via BD matmuls -> [b, t]
            def bd_contract(src, col, scale_tile, tag):
                wm = sm.tile([128, T], f32, tag="wm_" + tag, name="wm_" + tag)
                nc.vector.tensor_scalar(
                    out=wm,
                    in0=src,
                    scalar1=scale_tile[:, col : col + 1],
                    scalar2=None,
                    op0=alu.mult,
                )
                ps = sps.tile([B_LOC, T], f32, tag="ps", name="ps_" + tag, bufs=2)
                nc.tensor.matmul(ps, lhsT=bd_s, rhs=wm, start=True, stop=True)
                res = sm.tile([B_LOC, T], f32, tag="r_" + tag, name="r_" + tag)
                nc.vector.tensor_copy(res, ps)
                return res

            mx = bd_contract(w_s, 0, cxy_s, "mx")
            my = bd_contract(w_s, 1, cxy_s, "my")
            ca = bd_contract(w2q_s, 0, cov_s, "ca")
            cb = bd_contract(w2q_s, 1, cov_s, "cb")
            cc = bd_contract(w2q_s, 3, cov_s, "cc")

            def vt(tag):
                return sm.tile([B_LOC, T], f32, tag=tag, name=tag)

            mul, add, sub = alu.mult, alu.add, alu.subtract

            def tt(out, a, b, op):
                nc.vector.tensor_tensor(out=out, in0=a, in1=b, op=op)

            det = vt("det")
            tmp = vt("tmp")
            tt(det, ca, cc, mul)
            tt(tmp, cb, cb, mul)
            tt(det, det, tmp, sub)
            rdet = vt("rdet")
            nc.vector.reciprocal(rdet, det)

            i00 = vt("i00")  # = A coefficient (x^2)
            i01n = vt("i01n")  # +b/det (true inv offdiag is the negative)
            i11 = vt("i11")  # = C coefficient (y^2)
            tt(i00, cc, rdet, mul)
            tt(i01n, cb, rdet, mul)
            tt(i11, ca, rdet, mul)

            bq = vt("bq")  # B coefficient (xy) = -2*i01n
            nc.vector.tensor_scalar(
                out=bq, in0=i01n, scalar1=-2.0, scalar2=None, op0=mul
            )

            d1 = vt("d1")
            d2 = vt("d2")
            tt(d1, i00, mx, mul)  # i00*Mx
            tt(d2, i01n, my, mul)  # (b/det)*My
            dc = vt("dc")  # D coefficient (x) = 2*(d2 - d1)
            tt(dc, d2, d1, sub)
            nc.vector.tensor_scalar(out=dc, in0=dc, scalar1=2.0, scalar2=None, op0=mul)

            e1 = vt("e1")
            e2 = vt("e2")
            tt(e1, i11, my, mul)
            tt(e2, i01n, mx, mul)
            ec = vt("ec")  # E coefficient (y) = 2*(e2 - e1)
            tt(ec, e2, e1, sub)
            nc.vector.tensor_scalar(out=ec, in0=ec, scalar1=2.0, scalar2=None, op0=mul)

            # F' = i00 Mx^2 + i11 My^2 - 2 (b/det) Mx My + ln(det) + 2 ln(2pi)
            f1 = vt("f1")
            f2 = vt("f2")
            f3 = vt("f3")
            tt(f1, d1, mx, mul)
            tt(f2, e1, my, mul)
            tt(f3, d2, mx, mul)
            fc = vt("fc")
            tt(fc, f1, f2, add)
            nc.vector.tensor_scalar(
                out=f3, in0=f3, scalar1=-2.0, scalar2=None, op0=mul
            )
            tt(fc, fc, f3, add)
            ld = vt("ld")
            nc.scalar.activation(ld, det, func=act.Ln)
            tt(fc, fc, ld, add)
            nc.vector.tensor_scalar(
                out=fc, in0=fc, scalar1=log2pi2, scalar2=None, op0=add
            )

            # ---- assemble Coef [6, (b,t)] via partition-collapse DMAs ----
            coef = cpool.tile([6, B_LOC * T], f32, tag="coef")
            for c, src in enumerate([i00, bq, i11, dc, ec, fc]):
                nc.sync.dma_start(out=coef[c : c + 1, :], in_=src[:])

            # exact 3-way bf16 split of Coef; with the 2-way G split the
            # stacked K=36 bf16 matmul reproduces the fp32 product exactly
            # (all cross terms kept, fp32 PSUM accumulation).
            nbt = B_LOC * T
            ch_b = sm.tile([6, nbt], bf, tag="ch_b")
            nc.vector.tensor_copy(ch_b, coef)
            ch_f = sm.tile([6, nbt], f32, tag="ch_f")
            nc.vector.tensor_copy(ch_f, ch_b)
            r1 = sm.tile([6, nbt], f32, tag="r1")
            nc.vector.tensor_tensor(out=r1, in0=coef, in1=ch_f, op=alu.subtract)
            cm_b = sm.tile([6, nbt], bf, tag="cm_b")
            nc.vector.tensor_copy(cm_b, r1)
            cm_f = sm.tile([6, nbt], f32, tag="cm_f")
            nc.vector.tensor_copy(cm_f, cm_b)
            r2 = sm.tile([6, nbt], f32, tag="r2")
            nc.vector.tensor_tensor(out=r2, in0=r1, in1=cm_f, op=alu.subtract)
            cl_b = sm.tile([6, nbt], bf, tag="cl_b")
            nc.vector.tensor_copy(cl_b, r2)

            cstack = cpool.tile([36, nbt], bf, tag="cstack")
            for i, src in enumerate([ch_b, cm_b, cl_b, ch_b, cm_b, cl_b]):
                nc.sync.dma_start(out=cstack[6 * i : 6 * i + 6, :], in_=src[:])

            # small-stage PSUM no longer needed; free its banks for bpsum
            sps.release()

            # ---- big stage ----
            # 2 chunks per PSUM tile (4 banks): 4 matmuls -> 1 exp -> 1 DMA
            out_ap = out_d.ap()
            with tc.tile_pool(name="bpsum", bufs=2, space="PSUM") as bps:
                for pair in range(NCHUNK // 2):
                    q_ps = bps.tile([128, 2, B_LOC, T], f32, tag="q")
                    for ci in range(2):
                        chunk = pair * 2 + ci
                        for bg in range(2):
                            nc.tensor.matmul(
                                q_ps[:, ci, bg * 8 : (bg + 1) * 8, :],
                                lhsT=g_s[:, chunk * 128 : (chunk + 1) * 128],
                                rhs=cstack[:, bg * 512 : (bg + 1) * 512],
                                start=True,
                                stop=True,
                            )
                    o = big.tile([128, 2, B_LOC, T], f32, tag="o")
                    nc.scalar.activation(o, q_ps, func=act.Exp, scale=-0.5)
                    for ci in range(2):
                        dst = out_ap[:, pair * 2 + ci].rearrange("b p t -> p b t")
                        nc.sync.dma_start(out=dst, in_=o[:, ci])

    nc.compile()
    return nc


def _get_nc():
    if "nc" not in _CACHE:
        _CACHE["nc"] = _build_nc()
    return _CACHE["nc"]


def make_in_maps(cp_means, num_cps, cp_covariances):
    cp_means = np.asarray(cp_means, dtype=np.float32)
    cp_covariances = np.asarray(cp_covariances, dtype=np.float32)
    num_cps = np.asarray(num_cps)
    in_maps = []
    for c in range(NCORES):
        bsl = slice(c * B_LOC, (c + 1) * B_LOC)
        cxy = cp_means[:, bsl, :].transpose(1, 0, 2).reshape(128, 2)
        cab = cp_covariances[:, bsl].transpose(1, 0, 2, 3).reshape(128, 4)
        nrep = np.repeat(num_cps[bsl].astype(np.float32), K).reshape(1, 128)
        in_maps.append(
            {
                "cp_xy": np.ascontiguousarray(cxy),
                "cov_abc": np.ascontiguousarray(cab),
                "n_rep": np.ascontiguousarray(nrep),
            }
        )
    return in_maps


def kernel(cp_means, num_cps, cp_covariances):
    from concourse.bass_utils import run_bass_kernel_spmd

    nc = _get_nc()
    in_maps = make_in_maps(cp_means, num_cps, cp_covariances)
    res = run_bass_kernel_spmd(nc, in_maps, list(range(NCORES))).results
    out = np.concatenate(
        [res[i]["out"].reshape(B_LOC, W, H, T) for i in range(NCORES)], axis=0
    )
    return np.ascontiguousarray(out, dtype=np.float32)
